# revision 5
# baseline (speedup 1.0000x reference)
"""Trainium2 Bass kernel for nn_Block_CD (dual-stream patch-embed + attention).

Math per stream (x / y), tokens = (sample, l), l = 25 positions:
  xp  = conv3x3(img) + pos + conv_b           (im2col-ext matmul, K=52)
  xln = (xp - mu) * rstd                      (LN; g/b folded into qkv weights)
  qkv = (16 * qkv_w * g).T @ xln              (feature-major [768, tok])
  scores^T[(h,m),l] = Kbd_s^T @ q_s           (block-diag 4-head groups)
  E = exp(SCALE * scores); den = O^T @ E      (replicated over (h,d))
  av = Vbd_s^T @ E; avn = av / den
  out = proj^T @ avn + xp + (bp - pos - conv_b)
Output device layout: [512, B_loc*25] fp32, rearranged on host.
Sharding: pure data parallel, B=8192 over 8 cores.
"""
import sys
sys.path.insert(0, "/opt/trn_rl_repo")
import numpy as np
import ml_dtypes

import concourse.bass as bass
import concourse.mybir as mybir
import concourse.tile as tile
from concourse import bacc, bass_utils

bf16 = mybir.dt.bfloat16
f32 = mybir.dt.float32
AF = mybir.ActivationFunctionType
ALU = mybir.AluOpType

DIM = 256
HEADS = 8
HD = 32
L = 25
SCALE = HD ** -0.5
LN_EPS = 1e-5
NCORES = 8
B = 8192
B_LOC = B // NCORES

S_O = 64          # samples per outer chunk
S_I = 16          # samples per inner psum slice
N_I = S_I * L     # 400
TOK_O = S_O * L   # 1600

_CACHE = {}


def _to_bf16(a):
    return np.asarray(a, np.float32).astype(ml_dtypes.bfloat16)


def _host_prep(inputs):
    pos = np.asarray(inputs["pos_embed"], np.float32).reshape(L, DIM)
    ln_g = np.asarray(inputs["ln_g"], np.float32)
    ln_b = np.asarray(inputs["ln_b"], np.float32)

    def im2col_ext(img):
        p = np.pad(np.asarray(img, np.float32), ((0, 0), (0, 0), (1, 1), (1, 1)))
        Bn = img.shape[0]
        cols = np.empty((Bn, L, 52), np.float32)
        idx = 0
        for c in range(3):
            for di in range(3):
                for dj in range(3):
                    cols[:, :, idx] = p[:, c, di:di + 5, dj:dj + 5].reshape(Bn, L)
                    idx += 1
        cols[:, :, 27:] = np.eye(L, dtype=np.float32)[None]
        return cols  # [B, 25, 52]

    prep = {}
    for nm, ik, cw, cb, qw, pw, pb in (
        ("x", "x", "conv1_w", "conv1_b", "qkv_x_w", "proj_x_w", "proj_x_b"),
        ("y", "y", "conv2_w", "conv2_b", "qkv_y_w", "proj_y_w", "proj_y_b"),
    ):
        conv_w = np.asarray(inputs[cw], np.float32)
        conv_b = np.asarray(inputs[cb], np.float32)
        qkv_w = np.asarray(inputs[qw], np.float32)
        proj_w = np.asarray(inputs[pw], np.float32)
        proj_b = np.asarray(inputs[pb], np.float32)

        w_emb = np.empty((52, DIM), np.float32)
        w_emb[:27] = conv_w.reshape(DIM, 27).T
        w_emb[27:] = pos + conv_b[None, :]
        prep[f"wemb_{nm}"] = w_emb

        wq = (16.0 * qkv_w * ln_g[None, :]).T          # [256, 768]
        prep[f"wqkv_{nm}"] = np.concatenate([wq[0:128], wq[128:256]], axis=1)  # [128,1536]
        c = qkv_w @ ln_b
        assert np.abs(c).max() < 1e-6, "nonzero ln_b fold not supported"

        wp = proj_w.T                                   # [256, 256] lhsT
        prep[f"wproj_{nm}"] = np.concatenate([wp[0:128], wp[128:256]], axis=1)  # [128,512]

        fb = proj_b[:, None] - pos.T - conv_b[:, None]  # [256, 25]
        fbt = np.tile(fb, (1, S_I))                     # [256, 400]
        prep[f"fb_{nm}"] = np.concatenate([fbt[0:128], fbt[128:256]], axis=1)  # [128,800]

        prep[f"ic_{nm}"] = im2col_ext(inputs[ik])

    prep["ones_ln"] = np.full((128, 128), 1.0 / DIM, np.float32)
    O = np.zeros((100, 128), np.float32)
    for h in range(4):
        O[h * L:(h + 1) * L, h * HD:(h + 1) * HD] = 1.0
    prep["ones_den"] = O
    return prep


def _build_kernel(nc, tc, b_loc):
    import contextlib
    ctx = contextlib.ExitStack()
    n_chunk = b_loc // S_O
    n_sl = S_O // S_I

    dram = {}
    for nm in ("x", "y"):
        dram[f"ic_{nm}"] = nc.dram_tensor(f"ic_{nm}", [52, b_loc * L], bf16, kind="ExternalInput").ap()
        dram[f"wemb_{nm}"] = nc.dram_tensor(f"wemb_{nm}", [52, DIM], bf16, kind="ExternalInput").ap()
        dram[f"wqkv_{nm}"] = nc.dram_tensor(f"wqkv_{nm}", [128, 1536], bf16, kind="ExternalInput").ap()
        dram[f"wproj_{nm}"] = nc.dram_tensor(f"wproj_{nm}", [128, 512], bf16, kind="ExternalInput").ap()
        dram[f"fb_{nm}"] = nc.dram_tensor(f"fb_{nm}", [128, 2 * N_I], f32, kind="ExternalInput").ap()
    dram["ones_ln"] = nc.dram_tensor("ones_ln", [128, 128], bf16, kind="ExternalInput").ap()
    dram["ones_den"] = nc.dram_tensor("ones_den", [100, 128], bf16, kind="ExternalInput").ap()
    out_d = nc.dram_tensor("out", [2 * DIM, b_loc * L], f32, kind="ExternalOutput").ap()

    const = ctx.enter_context(tc.tile_pool(name="const", bufs=1))
    sb = ctx.enter_context(tc.tile_pool(name="sb", bufs=1))
    ps = ctx.enter_context(tc.tile_pool(name="ps", bufs=2, space="PSUM"))

    W = {}
    for nm in ("x", "y"):
        for key, shp, dt in (("emb", [52, DIM], bf16), ("qkv", [128, 1536], bf16),
                             ("proj", [128, 512], bf16), ("fb", [128, 2 * N_I], f32)):
            W[f"{key}_{nm}"] = const.tile(shp, dt, tag=f"{key}{nm}", name=f"{key}{nm}")
            nc.sync.dma_start(W[f"{key}_{nm}"][:, :], dram[f"w{key}_{nm}" if key != "fb" else f"fb_{nm}"])
    W["ones_ln"] = const.tile([128, 128], bf16, tag="ones_ln", name="ones_ln")
    nc.sync.dma_start(W["ones_ln"][:, :], dram["ones_ln"])
    W["ones_den"] = const.tile([100, 128], bf16, tag="ones_den", name="ones_den")
    nc.sync.dma_start(W["ones_den"][:, :], dram["ones_den"])
    eps256 = const.tile([128, 1], f32, tag="eps256", name="eps256")
    nc.vector.memset(eps256[:, :], 256.0 * LN_EPS)

    kbd, vbd = {}, {}
    for g in range(2):
        kbd[g] = const.tile([128, 100 * S_O], bf16, tag=f"kbd{g}", name=f"kbd{g}")
        nc.vector.memset(kbd[g][:, :], 0.0)
        vbd[g] = const.tile([128, 128 * S_O], bf16, tag=f"vbd{g}", name=f"vbd{g}")
        nc.vector.memset(vbd[g][:, :], 0.0)

    for ci in range(n_chunk):
        for nm in ("x", "y"):
            tok0 = ci * TOK_O
            # ---- embed ----
            ic = sb.tile([52, TOK_O], bf16, tag="ic", bufs=2)
            nc.sync.dma_start(ic[:, :], dram[f"ic_{nm}"][:, tok0:tok0 + TOK_O])
            xp = [sb.tile([128, TOK_O], bf16, tag=f"xp{t}", name=f"xp{t}") for t in range(2)]
            xpf = [sb.tile([128, TOK_O], f32, tag=f"xpf{t}", name=f"xpf{t}") for t in range(2)]
            sq = [sb.tile([128, TOK_O], bf16, tag=f"sq{t}", name=f"sq{t}") for t in range(2)]
            for t in range(2):
                for s in range(n_sl):
                    pt = ps.tile([128, N_I], f32, tag="mm")
                    nc.tensor.matmul(pt[:, :], W[f"emb_{nm}"][:, 128 * t:128 * (t + 1)],
                                     ic[:, s * N_I:(s + 1) * N_I], start=True, stop=True)
                    sl = slice(s * N_I, (s + 1) * N_I)
                    nc.scalar.activation(xpf[t][:, sl], pt[:, :], AF.Copy)
                    nc.vector.tensor_copy(xp[t][:, sl], pt[:, :])
                    nc.scalar.activation(sq[t][:, sl], pt[:, :], AF.Square)
            # ---- LN stats ----
            mu = sb.tile([128, TOK_O], f32, tag="mu")
            rs = sb.tile([128, TOK_O], f32, tag="rs")
            for s in range(n_sl):
                sl = slice(s * N_I, (s + 1) * N_I)
                pm = ps.tile([128, N_I], f32, tag="mm")
                nc.tensor.matmul(pm[:, :], W["ones_ln"][:, :], xp[0][:, sl], start=True, stop=False)
                nc.tensor.matmul(pm[:, :], W["ones_ln"][:, :], xp[1][:, sl], start=False, stop=True)
                pv = ps.tile([128, N_I], f32, tag="mm")
                nc.tensor.matmul(pv[:, :], W["ones_ln"][:, :], sq[0][:, sl], start=True, stop=False)
                nc.tensor.matmul(pv[:, :], W["ones_ln"][:, :], sq[1][:, sl], start=False, stop=True)
                nc.scalar.activation(mu[:, sl], pm[:, :], AF.Copy)
                t1 = sb.tile([128, N_I], f32, tag="t1")
                nc.scalar.activation(t1[:, :], pm[:, :], AF.Square)
                nc.vector.tensor_sub(t1[:, :], pv[:, :], t1[:, :])
                t2 = sb.tile([128, N_I], f32, tag="t2")
                nc.scalar.activation(t2[:, :], t1[:, :], AF.Sqrt, bias=eps256[:, 0:1], scale=256.0)
                nc.vector.reciprocal_approx_fast(rs[:, sl], t2[:, :])
            # ---- LN apply ----
            xln = [sb.tile([128, TOK_O], bf16, tag=f"xln{t}", name=f"xln{t}") for t in range(2)]
            for t in range(2):
                d = sb.tile([128, TOK_O], bf16, tag="lnd")
                nc.vector.tensor_sub(d[:, :], xp[t][:, :], mu[:, :])
                nc.vector.tensor_mul(xln[t][:, :], d[:, :], rs[:, :])
            # ---- qkv ----
            qkv = [sb.tile([128, TOK_O], bf16, tag=f"qkv{m}", name=f"qkv{m}") for m in range(4)]
            qkv += [sb.tile([128, 32 * S_O], bf16, tag=f"qkv{m}", name=f"qkv{m}") for m in (4, 5)]
            for m in (4, 5):
                nc.vector.memset(
                    qkv[m][:, :].rearrange("p (s l) -> p s l", l=32)[:, :, L:32], 0.0)
            for m in range(6):
                for s in range(n_sl):
                    sl = slice(s * N_I, (s + 1) * N_I)
                    pq = ps.tile([128, N_I], f32, tag="mm")
                    nc.tensor.matmul(pq[:, :], W[f"qkv_{nm}"][:, 128 * m:128 * (m + 1)],
                                     xln[0][:, sl], start=True, stop=False)
                    nc.tensor.matmul(pq[:, :], W[f"qkv_{nm}"][:, 768 + 128 * m:768 + 128 * (m + 1)],
                                     xln[1][:, sl], start=False, stop=True)
                    if m < 4:
                        nc.vector.tensor_copy(qkv[m][:, sl], pq[:, :])
                    else:
                        nc.vector.tensor_copy(
                            qkv[m][:, :].rearrange("p (s l) -> p s l", l=32)[:, s * S_I:(s + 1) * S_I, 0:L],
                            pq[:, :].rearrange("p (s l) -> p s l", l=L))
            # ---- attention ----
            vt = [sb.tile([128, 32 * S_O], bf16, tag=f"vt{g}", name=f"vt{g}") for g in range(2)]
            for g in range(2):
                nc.vector.transpose(vt[g][:, :], qkv[4 + g][:, :])
            for g in range(2):
                for h in range(4):
                    nc.sync.dma_start(
                        kbd[g][32 * h:32 * h + 32, :]
                        .rearrange("p (s m) -> p s m", m=100)[:, :, 25 * h:25 * h + 25],
                        qkv[2 + g][32 * h:32 * h + 32, :]
                        .rearrange("p (s m) -> p s m", m=L))
                    nc.sync.dma_start(
                        vbd[g][25 * h:25 * h + 25, :]
                        .rearrange("p (s d) -> p s d", d=128)[:, :, 32 * h:32 * h + 32],
                        vt[g][32 * h:32 * h + 25, :]
                        .rearrange("p (s d) -> p s d", d=32))
            ebuf = [sb.tile([128, L * S_O], bf16, tag=f"e{g}", name=f"e{g}") for g in range(2)]
            avn = [sb.tile([128, L * S_O], bf16, tag=f"avn{g}", name=f"avn{g}") for g in range(2)]
            for g in range(2):
                for s in range(n_sl):
                    sl = slice(s * N_I, (s + 1) * N_I)
                    sc = ps.tile([128, N_I], f32, tag="sc")
                    for j in range(S_I):
                        si = s * S_I + j
                        nc.tensor.matmul(
                            sc[0:100, j * L:(j + 1) * L],
                            kbd[g][:, 100 * si:100 * (si + 1)],
                            qkv[g][:, L * si:L * (si + 1)],
                            start=True, stop=True)
                    nc.scalar.activation(ebuf[g][0:100, sl], sc[0:100, :], AF.Exp, scale=SCALE)
                    dn = ps.tile([128, N_I], f32, tag="mm")
                    nc.tensor.matmul(dn[:, :], W["ones_den"][:, :], ebuf[g][0:100, sl],
                                     start=True, stop=True)
                    rden = sb.tile([128, N_I], f32, tag="rden", bufs=2)
                    nc.vector.reciprocal_approx_fast(rden[:, :], dn[:, :])
                    av = ps.tile([128, N_I], f32, tag="av")
                    for j in range(S_I):
                        si = s * S_I + j
                        nc.tensor.matmul(
                            av[:, j * L:(j + 1) * L],
                            vbd[g][0:100, 128 * si:128 * (si + 1)],
                            ebuf[g][0:100, L * si:L * (si + 1)],
                            start=True, stop=True)
                    nc.vector.tensor_mul(avn[g][:, sl], av[:, :], rden[:, :])
            # ---- proj + residual + out ----
            ob = 0 if nm == "x" else DIM
            for t in range(2):
                for s in range(n_sl):
                    sl = slice(s * N_I, (s + 1) * N_I)
                    pp = ps.tile([128, N_I], f32, tag="mm")
                    nc.tensor.matmul(pp[:, :], W[f"proj_{nm}"][:, 128 * t:128 * (t + 1)],
                                     avn[0][:, sl], start=True, stop=False)
                    nc.tensor.matmul(pp[:, :], W[f"proj_{nm}"][:, 256 + 128 * t:256 + 128 * (t + 1)],
                                     avn[1][:, sl], start=False, stop=True)
                    o1 = sb.tile([128, N_I], f32, tag="o1")
                    nc.vector.tensor_add(o1[:, :], pp[:, :], xpf[t][:, sl])
                    o2 = sb.tile([128, N_I], f32, tag="o2", bufs=2)
                    nc.vector.tensor_add(o2[:, :], o1[:, :],
                                         W[f"fb_{nm}"][:, N_I * t:N_I * (t + 1)])
                    nc.sync.dma_start(
                        out_d[ob + 128 * t: ob + 128 * (t + 1),
                              tok0 + s * N_I: tok0 + (s + 1) * N_I],
                        o2[:, :])
    ctx.close()


def _get_nc(b_loc):
    if b_loc in _CACHE:
        return _CACHE[b_loc]
    nc = bacc.Bacc("TRN2", target_bir_lowering=False, debug=False,
                   enable_asserts=False, num_devices=NCORES)
    with tile.TileContext(nc, trace_sim=False) as tc:
        _build_kernel(nc, tc, b_loc)
    nc.compile()
    bass.Bass.finalize(nc)
    _CACHE[b_loc] = nc
    return nc


def _in_maps(prep, b_loc, ncores):
    maps = []
    for c in range(ncores):
        s0 = c * b_loc
        m = {}
        for nm in ("x", "y"):
            ic = prep[f"ic_{nm}"][s0:s0 + b_loc].reshape(b_loc * L, 52).T
            m[f"ic_{nm}"] = _to_bf16(np.ascontiguousarray(ic))
            m[f"wemb_{nm}"] = _to_bf16(prep[f"wemb_{nm}"])
            m[f"wqkv_{nm}"] = _to_bf16(prep[f"wqkv_{nm}"])
            m[f"wproj_{nm}"] = _to_bf16(prep[f"wproj_{nm}"])
            m[f"fb_{nm}"] = prep[f"fb_{nm}"].astype(np.float32)
        m["ones_ln"] = _to_bf16(prep["ones_ln"])
        m["ones_den"] = _to_bf16(prep["ones_den"])
        maps.append(m)
    return maps


def kernel(**inputs):
    prep = _host_prep(inputs)
    nc = _get_nc(B_LOC)
    res = bass_utils.run_bass_kernel_spmd(nc, _in_maps(prep, B_LOC, NCORES),
                                          core_ids=list(range(NCORES)))
    outs = [res.results[c]["out"] for c in range(NCORES)]
    full = np.concatenate(
        [np.asarray(o, np.float32).reshape(2 * DIM, B_LOC, L).transpose(1, 0, 2)
         for o in outs], axis=0)
    return np.ascontiguousarray(full.reshape(B, 2 * DIM, 5, 5))


# revision 9
# speedup vs baseline: 16127.0458x; 16127.0458x over previous
"""Trainium2 Bass kernel for nn_Block_CD (dual-stream patch-embed + attention).

Math per stream (x / y), tokens = (sample, l), l = 25 positions:
  xp  = conv3x3(img) + pos + conv_b           (im2col-ext matmul, K=52)
  xln = (xp - mu) * rstd                      (LN; g/b folded into qkv weights)
  qkv = (16 * qkv_w * g).T @ xln              (feature-major [768, tok])
  scores^T[(h,m),l] = Kbd_s^T @ q_s           (block-diag 4-head groups)
  E = exp(SCALE * scores); den = O^T @ E      (replicated over (h,d))
  av = Vbd_s^T @ E; avn = av / den
  out = proj^T @ avn + xp + (bp - pos - conv_b)
Output device layout: [512, B_loc*25] fp32, rearranged on host.
Sharding: pure data parallel, B=8192 over 8 cores.
"""
import sys
sys.path.insert(0, "/opt/trn_rl_repo")
import numpy as np
import ml_dtypes

import concourse.bass as bass
import concourse.mybir as mybir
import concourse.tile as tile
from concourse import bacc, bass_utils

bf16 = mybir.dt.bfloat16
f32 = mybir.dt.float32
AF = mybir.ActivationFunctionType
ALU = mybir.AluOpType

DIM = 256
HEADS = 8
HD = 32
L = 25
SCALE = HD ** -0.5
LN_EPS = 1e-5
NCORES = 8
B = 8192
B_LOC = B // NCORES

S_O = 64          # samples per outer chunk
S_I = 16          # samples per inner psum slice
N_I = S_I * L     # 400
TOK_O = S_O * L   # 1600

_CACHE = {}


def _to_bf16(a):
    return np.asarray(a, np.float32).astype(ml_dtypes.bfloat16)


def _host_prep(inputs):
    pos = np.asarray(inputs["pos_embed"], np.float32).reshape(L, DIM)
    ln_g = np.asarray(inputs["ln_g"], np.float32)
    ln_b = np.asarray(inputs["ln_b"], np.float32)

    def im2col_ext(img):
        p = np.pad(np.asarray(img, np.float32), ((0, 0), (0, 0), (1, 1), (1, 1)))
        Bn = img.shape[0]
        cols = np.empty((Bn, L, 52), np.float32)
        idx = 0
        for c in range(3):
            for di in range(3):
                for dj in range(3):
                    cols[:, :, idx] = p[:, c, di:di + 5, dj:dj + 5].reshape(Bn, L)
                    idx += 1
        cols[:, :, 27:] = np.eye(L, dtype=np.float32)[None]
        return cols  # [B, 25, 52]

    prep = {}
    for nm, ik, cw, cb, qw, pw, pb in (
        ("x", "x", "conv1_w", "conv1_b", "qkv_x_w", "proj_x_w", "proj_x_b"),
        ("y", "y", "conv2_w", "conv2_b", "qkv_y_w", "proj_y_w", "proj_y_b"),
    ):
        conv_w = np.asarray(inputs[cw], np.float32)
        conv_b = np.asarray(inputs[cb], np.float32)
        qkv_w = np.asarray(inputs[qw], np.float32)
        proj_w = np.asarray(inputs[pw], np.float32)
        proj_b = np.asarray(inputs[pb], np.float32)

        w_emb = np.empty((52, DIM), np.float32)
        w_emb[:27] = conv_w.reshape(DIM, 27).T
        w_emb[27:] = pos + conv_b[None, :]
        prep[f"wemb_{nm}"] = w_emb

        wq = (16.0 * qkv_w * ln_g[None, :]).T          # [256, 768]
        prep[f"wqkv_{nm}"] = np.concatenate([wq[0:128], wq[128:256]], axis=1)  # [128,1536]
        c = qkv_w @ ln_b
        assert np.abs(c).max() < 1e-6, "nonzero ln_b fold not supported"

        wp = proj_w.T                                   # [256, 256] lhsT
        prep[f"wproj_{nm}"] = np.concatenate([wp[0:128], wp[128:256]], axis=1)  # [128,512]

        fb = proj_b[:, None] - pos.T - conv_b[:, None]  # [256, 25]
        fbt = np.tile(fb, (1, S_I))                     # [256, 400]
        prep[f"fb_{nm}"] = np.concatenate([fbt[0:128], fbt[128:256]], axis=1)  # [128,800]

        prep[f"ic_{nm}"] = im2col_ext(inputs[ik])

    prep["ones_ln"] = np.full((128, 128), 1.0 / DIM, np.float32)
    O = np.zeros((128, 128), np.float32)
    for h in range(4):
        O[h * HD:h * HD + L, h * HD:(h + 1) * HD] = 1.0
    prep["ones_den"] = O
    return prep


def _build_kernel(nc, tc, b_loc):
    import contextlib
    ctx = contextlib.ExitStack()
    n_chunk = b_loc // S_O
    n_sl = S_O // S_I

    dram = {}
    for nm in ("x", "y"):
        dram[f"ic_{nm}"] = nc.dram_tensor(f"ic_{nm}", [52, b_loc * L], bf16, kind="ExternalInput").ap()
        dram[f"wemb_{nm}"] = nc.dram_tensor(f"wemb_{nm}", [52, DIM], bf16, kind="ExternalInput").ap()
        dram[f"wqkv_{nm}"] = nc.dram_tensor(f"wqkv_{nm}", [128, 1536], bf16, kind="ExternalInput").ap()
        dram[f"wproj_{nm}"] = nc.dram_tensor(f"wproj_{nm}", [128, 512], bf16, kind="ExternalInput").ap()
        dram[f"fb_{nm}"] = nc.dram_tensor(f"fb_{nm}", [128, 2 * N_I], f32, kind="ExternalInput").ap()
    dram["ones_ln"] = nc.dram_tensor("ones_ln", [128, 128], bf16, kind="ExternalInput").ap()
    dram["ones_den"] = nc.dram_tensor("ones_den", [128, 128], bf16, kind="ExternalInput").ap()
    out_d = nc.dram_tensor("out", [2 * DIM, b_loc * L], f32, kind="ExternalOutput").ap()

    const = ctx.enter_context(tc.tile_pool(name="const", bufs=1))
    sb = ctx.enter_context(tc.tile_pool(name="sb", bufs=1))
    ps = ctx.enter_context(tc.tile_pool(name="ps", bufs=2, space="PSUM"))

    W = {}
    for nm in ("x", "y"):
        for key, shp, dt in (("emb", [52, DIM], bf16), ("qkv", [128, 1536], bf16),
                             ("proj", [128, 512], bf16), ("fb", [128, 2 * N_I], f32)):
            W[f"{key}_{nm}"] = const.tile(shp, dt, tag=f"{key}{nm}", name=f"{key}{nm}")
            nc.sync.dma_start(W[f"{key}_{nm}"][:, :], dram[f"w{key}_{nm}" if key != "fb" else f"fb_{nm}"])
    W["ones_ln"] = const.tile([128, 128], bf16, tag="ones_ln", name="ones_ln")
    nc.sync.dma_start(W["ones_ln"][:, :], dram["ones_ln"])
    W["ones_den"] = const.tile([128, 128], bf16, tag="ones_den", name="ones_den")
    nc.sync.dma_start(W["ones_den"][:, :], dram["ones_den"])
    eps256 = const.tile([128, 1], f32, tag="eps256", name="eps256")
    nc.vector.memset(eps256[:, :], 256.0 * LN_EPS)

    kbd, vbd = {}, {}
    for g in range(2):
        kbd[g] = const.tile([128, 128 * S_O], bf16, tag=f"kbd{g}", name=f"kbd{g}")
        nc.vector.memset(kbd[g][:, :], 0.0)
        vbd[g] = const.tile([128, 128 * S_O], bf16, tag=f"vbd{g}", name=f"vbd{g}")
        nc.vector.memset(vbd[g][:, :], 0.0)

    for ci in range(n_chunk):
        for nm in ("x", "y"):
            tok0 = ci * TOK_O
            # ---- embed ----
            ic = sb.tile([52, TOK_O], bf16, tag="ic", bufs=2)
            nc.sync.dma_start(ic[:, :], dram[f"ic_{nm}"][:, tok0:tok0 + TOK_O])
            xp = [sb.tile([128, TOK_O], bf16, tag=f"xp{t}", name=f"xp{t}") for t in range(2)]
            xpf = [sb.tile([128, TOK_O], f32, tag=f"xpf{t}", name=f"xpf{t}") for t in range(2)]
            sq = [sb.tile([128, TOK_O], bf16, tag=f"sq{t}", name=f"sq{t}") for t in range(2)]
            for t in range(2):
                for s in range(n_sl):
                    pt = ps.tile([128, N_I], f32, tag="mm", bufs=3)
                    nc.tensor.matmul(pt[:, :], W[f"emb_{nm}"][:, 128 * t:128 * (t + 1)],
                                     ic[:, s * N_I:(s + 1) * N_I], start=True, stop=True)
                    sl = slice(s * N_I, (s + 1) * N_I)
                    nc.scalar.activation(xpf[t][:, sl], pt[:, :], AF.Copy)
                    nc.vector.tensor_copy(xp[t][:, sl], pt[:, :])
                    nc.scalar.activation(sq[t][:, sl], pt[:, :], AF.Square)
            # ---- LN stats ----
            mu = sb.tile([128, TOK_O], f32, tag="mu")
            rs = sb.tile([128, TOK_O], f32, tag="rs")
            for s in range(n_sl):
                sl = slice(s * N_I, (s + 1) * N_I)
                pm = ps.tile([128, N_I], f32, tag="mm", bufs=3)
                nc.tensor.matmul(pm[:, :], W["ones_ln"][:, :], xp[0][:, sl], start=True, stop=False)
                nc.tensor.matmul(pm[:, :], W["ones_ln"][:, :], xp[1][:, sl], start=False, stop=True)
                pv = ps.tile([128, N_I], f32, tag="mm", bufs=3)
                nc.tensor.matmul(pv[:, :], W["ones_ln"][:, :], sq[0][:, sl], start=True, stop=False)
                nc.tensor.matmul(pv[:, :], W["ones_ln"][:, :], sq[1][:, sl], start=False, stop=True)
                nc.scalar.activation(mu[:, sl], pm[:, :], AF.Copy)
                t1 = sb.tile([128, N_I], f32, tag="t1")
                nc.scalar.activation(t1[:, :], pm[:, :], AF.Square)
                nc.vector.tensor_sub(t1[:, :], pv[:, :], t1[:, :])
                t2 = sb.tile([128, N_I], f32, tag="t2")
                nc.scalar.activation(t2[:, :], t1[:, :], AF.Sqrt, bias=eps256[:, 0:1], scale=256.0)
                nc.vector.reciprocal_approx_fast(rs[:, sl], t2[:, :])
            # ---- LN apply ----
            xln = [sb.tile([128, TOK_O], bf16, tag=f"xln{t}", name=f"xln{t}") for t in range(2)]
            for t in range(2):
                d = sb.tile([128, TOK_O], bf16, tag="lnd")
                nc.vector.tensor_sub(d[:, :], xp[t][:, :], mu[:, :])
                nc.vector.tensor_mul(xln[t][:, :], d[:, :], rs[:, :])
            # ---- qkv ----
            qkv = [sb.tile([128, TOK_O], bf16, tag=f"qkv{m}", name=f"qkv{m}") for m in range(4)]
            qkv += [sb.tile([128, 32 * S_O], bf16, tag=f"qkv{m}", name=f"qkv{m}") for m in (4, 5)]
            for m in (4, 5):
                nc.vector.memset(
                    qkv[m][:, :].rearrange("p (s l) -> p s l", l=32)[:, :, L:32], 0.0)
            for m in range(6):
                for s in range(n_sl):
                    sl = slice(s * N_I, (s + 1) * N_I)
                    pq = ps.tile([128, N_I], f32, tag="mm", bufs=3)
                    nc.tensor.matmul(pq[:, :], W[f"qkv_{nm}"][:, 128 * m:128 * (m + 1)],
                                     xln[0][:, sl], start=True, stop=False)
                    nc.tensor.matmul(pq[:, :], W[f"qkv_{nm}"][:, 768 + 128 * m:768 + 128 * (m + 1)],
                                     xln[1][:, sl], start=False, stop=True)
                    if m < 4:
                        nc.vector.tensor_copy(qkv[m][:, sl], pq[:, :])
                    else:
                        nc.vector.tensor_copy(
                            qkv[m][:, :].rearrange("p (s l) -> p s l", l=32)[:, s * S_I:(s + 1) * S_I, 0:L],
                            pq[:, :].rearrange("p (s l) -> p s l", l=L))
            # ---- attention ----
            vt = [sb.tile([128, 32 * S_O], bf16, tag=f"vt{g}", name=f"vt{g}") for g in range(2)]
            for g in range(2):
                nc.vector.transpose(vt[g][:, :], qkv[4 + g][:, :])
            for g in range(2):
                for h in range(4):
                    nc.scalar.activation(
                        kbd[g][32 * h:32 * h + 32, :]
                        .rearrange("p (s m) -> p s m", m=128)[:, :, 32 * h:32 * h + 25],
                        qkv[2 + g][32 * h:32 * h + 32, :]
                        .rearrange("p (s m) -> p s m", m=L), AF.Copy)
                    nc.vector.tensor_copy(
                        vbd[g][32 * h:32 * h + 25, :]
                        .rearrange("p (s d) -> p s d", d=128)[:, :, 32 * h:32 * h + 32],
                        vt[g][32 * h:32 * h + 25, :]
                        .rearrange("p (s d) -> p s d", d=32))
            ebuf = [sb.tile([128, L * S_O], bf16, tag=f"e{g}", name=f"e{g}") for g in range(2)]
            avn = [sb.tile([128, L * S_O], bf16, tag=f"avn{g}", name=f"avn{g}") for g in range(2)]
            for g in range(2):
                for s in range(n_sl):
                    sl = slice(s * N_I, (s + 1) * N_I)
                    sc = ps.tile([128, N_I], f32, tag="sc", bufs=3)
                    for j in range(S_I):
                        si = s * S_I + j
                        nc.tensor.matmul(
                            sc[0:128, j * L:(j + 1) * L],
                            kbd[g][:, 128 * si:128 * (si + 1)],
                            qkv[g][:, L * si:L * (si + 1)],
                            start=True, stop=True)
                    nc.scalar.activation(ebuf[g][:, sl], sc[:, :], AF.Exp, scale=SCALE)
                    dn = ps.tile([128, N_I], f32, tag="mm", bufs=3)
                    nc.tensor.matmul(dn[:, :], W["ones_den"][:, :], ebuf[g][:, sl],
                                     start=True, stop=True)
                    rden = sb.tile([128, N_I], f32, tag="rden", bufs=2)
                    nc.vector.reciprocal_approx_fast(rden[:, :], dn[:, :])
                    av = ps.tile([128, N_I], f32, tag="av")
                    for j in range(S_I):
                        si = s * S_I + j
                        nc.tensor.matmul(
                            av[:, j * L:(j + 1) * L],
                            vbd[g][:, 128 * si:128 * (si + 1)],
                            ebuf[g][:, L * si:L * (si + 1)],
                            start=True, stop=True)
                    nc.vector.tensor_mul(avn[g][:, sl], av[:, :], rden[:, :])
            # ---- proj + residual + out ----
            ob = 0 if nm == "x" else DIM
            for t in range(2):
                for s in range(n_sl):
                    sl = slice(s * N_I, (s + 1) * N_I)
                    pp = ps.tile([128, N_I], f32, tag="mm", bufs=3)
                    nc.tensor.matmul(pp[:, :], W[f"proj_{nm}"][:, 128 * t:128 * (t + 1)],
                                     avn[0][:, sl], start=True, stop=False)
                    nc.tensor.matmul(pp[:, :], W[f"proj_{nm}"][:, 256 + 128 * t:256 + 128 * (t + 1)],
                                     avn[1][:, sl], start=False, stop=True)
                    o1 = sb.tile([128, N_I], f32, tag="o1")
                    nc.vector.tensor_add(o1[:, :], pp[:, :], xpf[t][:, sl])
                    o2 = sb.tile([128, N_I], f32, tag="o2", bufs=2)
                    nc.vector.tensor_add(o2[:, :], o1[:, :],
                                         W[f"fb_{nm}"][:, N_I * t:N_I * (t + 1)])
                    nc.sync.dma_start(
                        out_d[ob + 128 * t: ob + 128 * (t + 1),
                              tok0 + s * N_I: tok0 + (s + 1) * N_I],
                        o2[:, :])
    ctx.close()


def _get_nc(b_loc):
    if b_loc in _CACHE:
        return _CACHE[b_loc]
    nc = bacc.Bacc("TRN2", target_bir_lowering=False, debug=False,
                   enable_asserts=False, num_devices=NCORES)
    with tile.TileContext(nc, trace_sim=False) as tc:
        _build_kernel(nc, tc, b_loc)
    nc.compile()
    bass.Bass.finalize(nc)
    _CACHE[b_loc] = nc
    return nc


def _in_maps(prep, b_loc, ncores):
    maps = []
    for c in range(ncores):
        s0 = c * b_loc
        m = {}
        for nm in ("x", "y"):
            ic = prep[f"ic_{nm}"][s0:s0 + b_loc].reshape(b_loc * L, 52).T
            m[f"ic_{nm}"] = _to_bf16(np.ascontiguousarray(ic))
            m[f"wemb_{nm}"] = _to_bf16(prep[f"wemb_{nm}"])
            m[f"wqkv_{nm}"] = _to_bf16(prep[f"wqkv_{nm}"])
            m[f"wproj_{nm}"] = _to_bf16(prep[f"wproj_{nm}"])
            m[f"fb_{nm}"] = prep[f"fb_{nm}"].astype(np.float32)
        m["ones_ln"] = _to_bf16(prep["ones_ln"])
        m["ones_den"] = _to_bf16(prep["ones_den"])
        maps.append(m)
    return maps


def kernel(**inputs):
    prep = _host_prep(inputs)
    nc = _get_nc(B_LOC)
    res = bass_utils.run_bass_kernel_spmd(nc, _in_maps(prep, B_LOC, NCORES),
                                          core_ids=list(range(NCORES)))
    outs = [res.results[c]["out"] for c in range(NCORES)]
    full = np.concatenate(
        [np.asarray(o, np.float32).reshape(2 * DIM, B_LOC, L).transpose(1, 0, 2)
         for o in outs], axis=0)
    return np.ascontiguousarray(full.reshape(B, 2 * DIM, 5, 5))
